# revision 7
# baseline (speedup 1.0000x reference)
"""Distributed AttentionAutoEncoder kernel for 8 TRN2 NeuronCores (v5).

Reference computation (fp32):
    Q = W_q @ X ; K = W_v @ X ; V = W_k @ X          (d=2048, n=8192)
    S = (Q @ K.T) / sqrt(d) ; Z = softmax(S, -1) ; A = Z @ V

Reformulation: S = W_q @ G @ W_v.T / sqrt(d) with G = X @ X.T (AllReduce
over the n contraction), sharded by ROWS of S (i-sharding):
    Rt = G @ (W_q.T/sqrt(d))[:, i_c]        (2048 x 256)
    S[i_c, :] = Rt.T @ W_v.T                (256 x 2048)
softmax locally on 2 row-tiles, P transposed to (j,i), one fp16
AllGather, then transpose-free A = P @ V.

v3 replaces v2's 3-pass fp16 hi/lo matmuls in the whole S chain
(G / Rt / S) with single-pass float32r: measured on this hw fp32r runs
at ~the fp16 moving rate for free dims >= 256 with ~14-bit-per-product
precision, which keeps the predicted score noise (dS rms ~ 1.2) well
inside the softmax flip budget (fro_rel ~ 7e-3 < 2e-2 gate).  Storage
stays fp32 end-to-end; matmul operands are .bitcast(float32r) views.
V / P / A remain single-pass fp16 (error enters A only linearly).
"""

import numpy as np

import concourse.bacc as bacc
import concourse.mybir as mybir
import concourse.tile as tile
from concourse.masks import make_identity

P = 128
FP16 = mybir.dt.float16
FP32 = mybir.dt.float32
FP32R = mybir.dt.float32r
AF = mybir.ActivationFunctionType

D_FULL = 2048
N_FULL = 8192
NCORES = 8


def build(D=D_FULL, NL=N_FULL // NCORES, NC=NCORES, stop_after=None,
          nchunk=8, mock_coll=False, npacks=8):
    """Build the SPMD Bass program (identical on every core)."""
    JS = D // NC          # i-rows of S owned by this core
    IT = JS // P          # i-tiles per core (2)
    nT = NL // P          # n-tiles per core
    dT = D // P           # d-tiles
    CB = min(512, D)      # column-block width over d
    NBS = min(512, NL)    # column-block width over n
    KB = D // CB          # column blocks of d
    NB = NL // NBS        # column blocks of n
    assert NL % NBS == 0 and D % CB == 0

    nc = bacc.Bacc("TRN2", target_bir_lowering=False, debug=False,
                   num_devices=NC)

    def r(ap):
        return ap.bitcast(FP32R)

    # ------------- I/O -------------
    xt = nc.dram_tensor("xt", [NL, D], FP32R, kind="ExternalInput")
    xn_hi = nc.dram_tensor("xn_hi", [D, NL], FP16, kind="ExternalInput")
    wkt_hi = nc.dram_tensor("wkt_hi", [dT // 4, dT, P, 4 * P], FP16,
                            kind="ExternalInput")
    wqts = nc.dram_tensor("wqts", [D, JS], FP32R, kind="ExternalInput")
    wvs = nc.dram_tensor("wvs", [KB, P, dT * CB], FP32R,
                         kind="ExternalInput")
    t1s = nc.dram_tensor("t1s", [JS, D], FP32, kind="ExternalInput")
    a_out = nc.dram_tensor("a_out", [D, NL], FP32, kind="ExternalOutput")

    with tile.TileContext(nc) as tc:
        with tc.tile_pool(name="dram", bufs=1, space="DRAM") as dpool:
            NCHUNK = min(nchunk, dT)
            CM = dT // NCHUNK          # m-tiles per compute chunk
            # G is symmetric: chunk c only stores columns >= kb0(c)*CB.
            # Chunks are PACKED pairwise into NPACK buffers so the AllReduce
            # is 2 calls (collective per-call floor dominates here); packed
            # cols keep their natural kb position minus the pack's base, so
            # pack p covers cols >= pb0(p)*CB and chunk c's unstored
            # (below-base) columns are junk that no reader touches.
            kb0 = [next(kb for kb in range(KB)
                        if (kb + 1) * CB > c * CM * P)
                   for c in range(NCHUNK)]
            assert NCHUNK % npacks == 0
            _pc = NCHUNK // npacks
            PACKS = tuple(tuple(range(p * _pc, (p + 1) * _pc))
                          for p in range(npacks))
            pack_of = {c: p for p, cs in enumerate(PACKS) for c in cs}
            pos_in = {c: i for cs in PACKS for i, c in enumerate(cs)}
            pb0 = [kb0[cs[0]] for cs in PACKS]
            _ashr = "Local" if mock_coll else "Shared"
            g_in = [dpool.tile([len(cs) * CM * P, (KB - pb0[p]) * CB], FP32,
                               name=f"g_in{p}") for p, cs in enumerate(PACKS)]
            g_out = [dpool.tile([len(cs) * CM * P, (KB - pb0[p]) * CB], FP32,
                                name=f"g_out{p}",
                                addr_space=_ashr)
                     for p, cs in enumerate(PACKS)]
            pt_in = [dpool.tile([D // 2, JS], FP16, name=f"pt_in{h}")
                     for h in range(2)]
            pt_out = [dpool.tile([NC, D // 2, JS], FP16, name=f"pt_out{h}",
                                 addr_space=_ashr) for h in range(2)]
            v_park = dpool.tile([D, NL], FP16, name="v_park")

            with tc.tile_pool(name="idt", bufs=1) as idt_pool, \
                 tc.tile_pool(name="stats", bufs=1) as stats_pool, \
                 tc.tile_pool(name="rt", bufs=1) as rt_pool:

                identT = idt_pool.tile([P, P], FP16, name="identT")
                make_identity(nc, identT)
                ident32f = idt_pool.tile([P, P], FP32, name="ident32f")
                make_identity(nc, ident32f)
                nident = idt_pool.tile([P, P], FP32, name="nident")
                make_identity(nc, nident)
                nc.scalar.mul(nident, nident, float(NL))
                ident32 = idt_pool.tile([P, P], FP32R, name="ident32")
                nc.scalar.copy(ident32, ident32f)
                mx = stats_pool.tile([P, IT], FP32, name="mx")
                negm = stats_pool.tile([P, IT], FP32, name="negm")
                ssum = stats_pool.tile([P, IT], FP32, name="ssum")
                recip = stats_pool.tile([P, IT], FP32, name="recip")

                # ---------------- Phase 1: G = X X^T (fp32r), then V -------
                with tc.tile_pool(name="xt", bufs=1) as xt_pool, \
                     tc.tile_pool(name="xn", bufs=1) as xn_pool, \
                     tc.tile_pool(name="vsb", bufs=1) as v_pool:
                    v_sb = [v_pool.tile([P, NL], FP16, name=f"v{iv}")
                            for iv in range(dT)]

                    xts = []
                    for n in range(nT):
                        t = xt_pool.tile([P, D], FP32R, name=f"xt{n}")
                        nc.sync.dma_start(out=t,
                                          in_=xt[n * P:(n + 1) * P, :])
                        xts.append(t)

                    xn_sb = []
                    for k in range(dT):
                        t = xn_pool.tile([P, NL], FP16, name=f"xn{k}")
                        nc.sync.dma_start(out=t, in_=xn_hi[k * P:(k + 1) * P, :])
                        xn_sb.append(t)

                    # G matmuls: G[m,k] = sum_n XT[n,m] * XT[n,k]
                    with tc.tile_pool(name="gstg", bufs=8) as gstg_pool, \
                         tc.tile_pool(name="wk", bufs=4) as wk_pool:
                      wk_pre = []
                      if stop_after not in ("g", "ar"):
                          for k in range(2):
                              wt = wk_pool.tile([P, 4 * P], FP16,
                                                name="wk_t", tag="wk_t")
                              nc.scalar.dma_start(out=wt, in_=wkt_hi[0, k])
                              wk_pre.append(wt)
                      with tc.tile_pool(name="gps", bufs=8,
                                        space="PSUM") as gps_pool:
                        for m in range(dT):
                            ms = slice(m * P, (m + 1) * P)
                            c = m // CM
                            pk = pack_of[c]
                            prow0 = (pos_in[c] * CM + m % CM) * P
                            msl = slice(prow0, prow0 + P)
                            for kb in range(kb0[c], KB):
                                ks = slice(kb * CB, (kb + 1) * CB)
                                ps = gps_pool.tile([P, CB], FP32, name="g_ps",
                                                   tag="g_ps")
                                for n in range(nT):
                                    nc.tensor.matmul(ps, xts[n][:, ms],
                                                     xts[n][:, ks],
                                                     start=(n == 0),
                                                     stop=(n == nT - 1))
                                stg = gstg_pool.tile([P, CB], FP32,
                                                     name="g_stg", tag="g_stg")
                                nc.scalar.copy(stg, ps)
                                if kb == m // 4:
                                    do = (m % 4) * P
                                    nc.vector.tensor_sub(
                                        stg[:, do:do + P], ps[:, do:do + P],
                                        nident)
                                kpk = slice((kb - pb0[pk]) * CB,
                                            (kb - pb0[pk] + 1) * CB)
                                nc.sync.dma_start(out=g_in[pk][msl, kpk],
                                                  in_=stg)
                            # AllReduce a pack as soon as its rows are done
                            if (m % CM == CM - 1 and c == PACKS[pk][-1]
                                    and stop_after not in ("g",)):
                                if mock_coll:
                                    nc.sync.dma_start(out=g_out[pk][:, :],
                                                      in_=g_in[pk][:, :])
                                else:
                                    nc.gpsimd.collective_compute(
                                        "AllReduce", mybir.AluOpType.add,
                                        replica_groups=[list(range(NC))],
                                        ins=[g_in[pk].opt()],
                                        outs=[g_out[pk].opt()])

                      # V = W_k @ X (single-pass fp16), k-outer streamed WkT
                      with tc.tile_pool(name="vps", bufs=4 * NB,
                                        space="PSUM") as vps_pool:
                          for ivg in range(0, dT if stop_after not in
                                           ("g", "ar") else 0, 4):
                            pss = {}
                            for j in range(4):
                                for nb in range(NB):
                                    pss[(j, nb)] = vps_pool.tile(
                                        [P, NBS], FP32, name="v_ps",
                                        tag="v_ps")
                            for k in range(dT):
                                if ivg == 0 and k < len(wk_pre):
                                    wt = wk_pre[k]
                                else:
                                    wt = wk_pool.tile([P, 4 * P], FP16,
                                                      name="wk_t", tag="wk_t")
                                    nc.scalar.dma_start(
                                        out=wt, in_=wkt_hi[ivg // 4, k])
                                for j in range(4):
                                    for nb in range(NB):
                                        ns = slice(nb * NBS, (nb + 1) * NBS)
                                        nc.tensor.matmul(
                                            pss[(j, nb)],
                                            wt[:, j * P:(j + 1) * P],
                                            xn_sb[k][:, ns],
                                            start=(k == 0),
                                            stop=(k == dT - 1))
                            for j in range(4):
                                iv = ivg + j
                                for nb in range(NB):
                                    ns = slice(nb * NBS, (nb + 1) * NBS)
                                    nc.vector.tensor_copy(
                                        out=v_sb[iv][:, ns],
                                        in_=pss[(j, nb)])
                    for iv in range(dT if stop_after not in ("g", "ar")
                                    else 0):
                        nc.sync.dma_start(
                            out=v_park[iv * P:(iv + 1) * P, :],
                            in_=v_sb[iv])

                if stop_after in ("g", "ar", "v"):
                    # still need an output write so a_out is defined
                    dummy = rt_pool.tile([P, NL], FP32, name="dummy_out")
                    nc.vector.memset(dummy, 0.0)
                    for i in range(dT):
                        nc.sync.dma_start(out=a_out[i * P:(i + 1) * P, :],
                                          in_=dummy)
                else:
                    # -------- Phase 2: Rt = G @ (W_q^T/sqrt(d))[:, i_c] ----
                    rts = []
                    with tc.tile_pool(name="gsb", bufs=1) as g_pool, \
                         tc.tile_pool(name="wq", bufs=1) as wq_pool:

                        wqt = []
                        for k in range(dT):
                            h = wq_pool.tile([P, JS], FP32R, name=f"wq{k}")
                            nc.scalar.dma_start(
                                out=h, in_=wqts[k * P:(k + 1) * P, :])
                            wqt.append(h)

                        g_sb = []
                        with tc.tile_pool(name="mirps", bufs=4,
                                          space="PSUM") as mir_pool:
                            for k in range(dT):
                                ck = k // CM
                                pk = pack_of[ck]
                                nsto = D - kb0[ck] * CB
                                pcol = (kb0[ck] - pb0[pk]) * CB
                                prow = (pos_in[ck] * CM + k % CM) * P
                                g = g_pool.tile([P, D], FP32R, name=f"g{k}")
                                nc.scalar.dma_start(
                                    out=g[:, D - nsto:],
                                    in_=g_out[pk][prow:prow + P,
                                                  pcol:pcol + nsto]
                                    .bitcast(FP32R))
                                # mirror below-diagonal: G[k,q] = G[q,k]^T
                                kcs = slice(k * P, (k + 1) * P)
                                for q in range(kb0[ck] * CB // P):
                                    qcs = slice(q * P, (q + 1) * P)
                                    mp = mir_pool.tile([P, P], FP32R,
                                                       name="mir_ps",
                                                       tag="mir_ps")
                                    nc.tensor.transpose(mp[:, :],
                                                        g_sb[q][:, kcs],
                                                        ident32[:, :])
                                    nc.vector.tensor_copy(out=g[:, qcs],
                                                          in_=mp)
                                g_sb.append(g)

                        # Rt[m, i] = sum_k G[k, m] * wqts[k, i]
                        MG = min(8, dT)
                        with tc.tile_pool(name="tps", bufs=MG,
                                          space="PSUM") as tps_pool:
                            for mg in range(0, dT, MG):
                                pss = []
                                for m in range(mg, mg + MG):
                                    pss.append(tps_pool.tile(
                                        [P, JS], FP32, name="t_ps",
                                        tag="t_ps"))
                                for k in range(dT):
                                    for j, m in enumerate(
                                            range(mg, mg + MG)):
                                        ms = slice(m * P, (m + 1) * P)
                                        nc.tensor.matmul(
                                            pss[j], g_sb[k][:, ms],
                                            wqt[k][:, :],
                                            start=(k == 0),
                                            stop=(k == dT - 1))
                                for j, m in enumerate(range(mg, mg + MG)):
                                    h = rt_pool.tile([P, JS], FP32R,
                                                     name=f"rt{m}")
                                    nc.scalar.copy(h, pss[j])
                                    rts.append(h)

                    # -------- Phase 3: S[i_c, :] = Rt^T @ W_v^T ------------
                    with tc.tile_pool(name="s32", bufs=1) as s32_pool, \
                         tc.tile_pool(name="psb", bufs=1) as p_pool, \
                         tc.tile_pool(name="wv", bufs=2) as wv_pool, \
                         tc.tile_pool(name="v2", bufs=1) as v2_pool, \
                         tc.tile_pool(name="sps", bufs=4,
                                      space="PSUM") as sps_pool:
                        s_sb = [s32_pool.tile([P, D], FP32, name=f"s{it}")
                                for it in range(IT)]
                        for it in range(IT):
                            nc.scalar.dma_start(
                                out=s_sb[it],
                                in_=t1s[it * P:(it + 1) * P, :])
                        for jb in range(KB if stop_after != "rt" else 0):
                            wv_t = wv_pool.tile([P, dT * CB], FP32R,
                                                name="wv_t", tag="wv_t")
                            nc.scalar.dma_start(out=wv_t, in_=wvs[jb])
                            for it in range(IT):
                                isl = slice(it * P, (it + 1) * P)
                                ps = sps_pool.tile([P, CB], FP32, name="s_ps",
                                                   tag="s_ps")
                                for m in range(dT):
                                    msl = slice(m * CB, (m + 1) * CB)
                                    nc.tensor.matmul(
                                        ps, rts[m][:, isl],
                                        wv_t[:, msl],
                                        start=(m == 0),
                                        stop=(m == dT - 1))
                                jsl = slice(jb * CB, (jb + 1) * CB)
                                nc.vector.tensor_add(
                                    s_sb[it][:, jsl], ps, s_sb[it][:, jsl])

                        v_sb2 = []
                        if stop_after not in ("rt", "s", "ag"):
                            for iv in range(dT):
                                t = v2_pool.tile([P, NL], FP16,
                                                 name=f"v2_{iv}")
                                nc.scalar.dma_start(
                                    out=t,
                                    in_=v_park[iv * P:(iv + 1) * P, :])
                                v_sb2.append(t)

                        # -------- Phase 4: softmax rows + P^T + AllGather --
                        if stop_after not in ("rt", "s"):
                            with tc.tile_pool(name="ptl", bufs=4) as ptl_pool, \
                                 tc.tile_pool(name="ptps", bufs=4,
                                              space="PSUM") as ptps_pool:
                                pn = []
                                for it in range(IT):
                                    itc = slice(it, it + 1)
                                    nc.vector.reduce_max(
                                        mx[:, itc], s_sb[it],
                                        axis=mybir.AxisListType.X)
                                    nc.scalar.mul(negm[:, itc], mx[:, itc],
                                                  -1.0)
                                    pt = p_pool.tile([P, D], FP16,
                                                     name=f"p{it}")
                                    nc.scalar.activation(
                                        pt, s_sb[it], AF.Exp,
                                        bias=negm[:, itc], scale=1.0,
                                        accum_out=ssum[:, itc])
                                    nc.vector.reciprocal(recip[:, itc],
                                                         ssum[:, itc])
                                    pnt = p_pool.tile([P, D], FP16,
                                                      name=f"pn{it}")
                                    nc.vector.tensor_scalar_mul(
                                        pnt, pt, recip[:, itc])
                                    pn.append(pnt)
                                for half in range(2):
                                    for jt in range(half * dT // 2,
                                                    (half + 1) * dT // 2):
                                        jcs = slice(jt * P, (jt + 1) * P)
                                        jloc = jt - half * dT // 2
                                        ptl = ptl_pool.tile([P, JS], FP16,
                                                            name="ptl",
                                                            tag="ptl")
                                        for it in range(IT):
                                            mp = ptps_pool.tile([P, P], FP16,
                                                                name="pt_ps",
                                                                tag="pt_ps")
                                            nc.tensor.transpose(
                                                mp, pn[it][:, jcs], identT)
                                            nc.vector.tensor_copy(
                                                out=ptl[:,
                                                        it * P:(it + 1) * P],
                                                in_=mp)
                                        nc.sync.dma_start(
                                            out=pt_in[half][
                                                jloc * P:(jloc + 1) * P, :],
                                            in_=ptl)
                                    if mock_coll:
                                        for rr in range(NC):
                                            nc.sync.dma_start(
                                                out=pt_out[half][rr, :, :],
                                                in_=pt_in[half][:, :])
                                    else:
                                        nc.gpsimd.collective_compute(
                                            "AllGather",
                                            mybir.AluOpType.bypass,
                                            replica_groups=[list(range(NC))],
                                            ins=[pt_in[half].opt()],
                                            outs=[pt_out[half].opt()])

                    # -------- Phase 5: A = P @ V (transpose-free) ----------
                    if stop_after not in ("rt", "s", "ag"):
                        with tc.tile_pool(name="ptb", bufs=2) as ptb_pool, \
                             tc.tile_pool(name="asb", bufs=2) as a_pool, \
                             tc.tile_pool(name="aps", bufs=3,
                                          space="PSUM") as aps_pool:
                            v_sb = v_sb2
                            for rr in range(NC):
                                ptb = []
                                for jt in range(dT):
                                    half = jt // (dT // 2)
                                    jloc = jt - half * dT // 2
                                    t = ptb_pool.tile([P, JS], FP16,
                                                      name="ptb",
                                                      tag=f"ptb{jt}")
                                    nc.scalar.dma_start(
                                        out=t,
                                        in_=pt_out[half][
                                            rr, jloc * P:(jloc + 1) * P, :])
                                    ptb.append(t)
                                for isub in range(IT):
                                    i = rr * IT + isub
                                    isl = slice(isub * P, (isub + 1) * P)
                                    aps = aps_pool.tile([P, NL], FP32,
                                                        name="a_ps",
                                                        tag="a_ps")
                                    for jt in range(dT):
                                        for nb in range(NB):
                                            ns = slice(nb * NBS,
                                                       (nb + 1) * NBS)
                                            nc.tensor.matmul(
                                                aps[:, ns], ptb[jt][:, isl],
                                                v_sb[jt][:, ns],
                                                start=(jt == 0),
                                                stop=(jt == dT - 1))
                                    asb = a_pool.tile([P, NL], FP32,
                                                      name="a_sb", tag="a_sb")
                                    nc.vector.tensor_copy(out=asb, in_=aps)
                                    nc.sync.dma_start(
                                        out=a_out[i * P:(i + 1) * P, :],
                                        in_=asb)

    nc.compile()
    return nc


def prepare_inputs(X_t, W_q, W_k, W_v, NC=NCORES):
    """Host-side sharding + layout packing.  Returns in_maps for SPMD."""
    D, N = X_t.shape
    NL = N // NC
    JS = D // NC
    sc = np.float32(1.0) / np.sqrt(np.float32(D))
    dT = D // 128
    P_ = 128
    CB = 512
    KB = D // CB

    wkt_hi = np.ascontiguousarray(W_k.T.astype(np.float16))
    wkt_hi = np.ascontiguousarray(
        wkt_hi.reshape(dT, P_, dT // 4, 4 * P_).transpose(2, 0, 1, 3))

    wqts_full = np.ascontiguousarray(W_q.T.astype(np.float32) * sc)
    wvt = W_v.T.astype(np.float32)            # [D, D] = Wv^T
    # wvs[jb][p, m*CB+j] = Wv^T[m*128+p, jb*CB+j]
    wvs = np.ascontiguousarray(
        wvt.reshape(dT, P_, KB, CB).transpose(2, 1, 0, 3)
        .reshape(KB, P_, dT * CB))

    # weight folding: S = Wq G Wv^T sc = Wq (G - N I) Wv^T sc + N sc Wq Wv^T;
    # the data-independent second term is precomputed here (host), the device
    # computes only the (G - N I) chain.
    t1_full = (np.float32(N) * wqts_full.T @ W_v.T.astype(np.float32))
    in_maps = []
    for c in range(NC):
        xc = np.ascontiguousarray(X_t[:, c * NL:(c + 1) * NL]
                                  .astype(np.float32))
        in_maps.append({
            "xt": np.ascontiguousarray(xc.T),
            "xn_hi": np.ascontiguousarray(xc.astype(np.float16)),
            "wkt_hi": wkt_hi,
            "wqts": np.ascontiguousarray(wqts_full[:, c * JS:(c + 1) * JS]),
            "wvs": wvs,
            "t1s": np.ascontiguousarray(t1_full[c * JS:(c + 1) * JS, :]),
        })
    return in_maps


_CACHED_NC = None


def _get_nc():
    global _CACHED_NC
    if _CACHED_NC is None:
        _CACHED_NC = build()
    return _CACHED_NC


def run(X_t, W_q, W_k, W_v, trace=False):
    from concourse.bass_utils import run_bass_kernel_spmd
    nc = _get_nc()
    in_maps = prepare_inputs(X_t, W_q, W_k, W_v)
    res = run_bass_kernel_spmd(nc, in_maps, core_ids=list(range(NCORES)),
                               trace=trace)
    A = np.concatenate([res.results[c]["a_out"] for c in range(NCORES)],
                       axis=1)
    return A, res


def kernel(X_t, W_q, W_k, W_v):
    X_t = np.asarray(X_t)
    W_q = np.asarray(W_q)
    W_k = np.asarray(W_k)
    W_v = np.asarray(W_v)
    A, _ = run(X_t, W_q, W_k, W_v, trace=False)
    return A.astype(np.float32)


# revision 10
# speedup vs baseline: 1.2008x; 1.2008x over previous
"""Distributed AttentionAutoEncoder kernel for 8 TRN2 NeuronCores (v5).

Reference computation (fp32):
    Q = W_q @ X ; K = W_v @ X ; V = W_k @ X          (d=2048, n=8192)
    S = (Q @ K.T) / sqrt(d) ; Z = softmax(S, -1) ; A = Z @ V

Reformulation: S = W_q @ G @ W_v.T / sqrt(d) with G = X @ X.T (AllReduce
over the n contraction), sharded by ROWS of S (i-sharding):
    Rt = G @ (W_q.T/sqrt(d))[:, i_c]        (2048 x 256)
    S[i_c, :] = Rt.T @ W_v.T                (256 x 2048)
softmax locally on 2 row-tiles, P transposed to (j,i), one fp16
AllGather, then transpose-free A = P @ V.

v3 replaces v2's 3-pass fp16 hi/lo matmuls in the whole S chain
(G / Rt / S) with single-pass float32r: measured on this hw fp32r runs
at ~the fp16 moving rate for free dims >= 256 with ~14-bit-per-product
precision, which keeps the predicted score noise (dS rms ~ 1.2) well
inside the softmax flip budget (fro_rel ~ 7e-3 < 2e-2 gate).  Storage
stays fp32 end-to-end; matmul operands are .bitcast(float32r) views.
V / P / A remain single-pass fp16 (error enters A only linearly).
"""

import numpy as np

import concourse.bacc as bacc
import concourse.mybir as mybir
import concourse.tile as tile
from concourse.masks import make_identity

P = 128
FP16 = mybir.dt.float16
FP32 = mybir.dt.float32
FP32R = mybir.dt.float32r
AF = mybir.ActivationFunctionType

D_FULL = 2048
N_FULL = 8192
NCORES = 8


def build(D=D_FULL, NL=N_FULL // NCORES, NC=NCORES, stop_after=None,
          nchunk=8, mock_coll=False, npacks=8):
    """Build the SPMD Bass program (identical on every core)."""
    JS = D // NC          # i-rows of S owned by this core
    IT = JS // P          # i-tiles per core (2)
    nT = NL // P          # n-tiles per core
    dT = D // P           # d-tiles
    CB = min(512, D)      # column-block width over d
    NBS = min(512, NL)    # column-block width over n
    KB = D // CB          # column blocks of d
    NB = NL // NBS        # column blocks of n
    assert NL % NBS == 0 and D % CB == 0

    nc = bacc.Bacc("TRN2", target_bir_lowering=False, debug=False,
                   num_devices=NC)

    def r(ap):
        return ap.bitcast(FP32R)

    # ------------- I/O -------------
    xt = nc.dram_tensor("xt", [NL, D], FP32R, kind="ExternalInput")
    xn_hi = nc.dram_tensor("xn_hi", [D, NL], FP16, kind="ExternalInput")
    wkt_hi = nc.dram_tensor("wkt_hi", [dT // 4, dT, P, 4 * P], FP16,
                            kind="ExternalInput")
    wqts = nc.dram_tensor("wqts", [D, JS], FP32R, kind="ExternalInput")
    wvs = nc.dram_tensor("wvs", [KB, P, dT * CB], FP32R,
                         kind="ExternalInput")
    t1s = nc.dram_tensor("t1s", [JS, D], FP32, kind="ExternalInput")
    a_out = nc.dram_tensor("a_out", [D, NL], FP32, kind="ExternalOutput")

    with tile.TileContext(nc) as tc:
        with tc.tile_pool(name="dram", bufs=1, space="DRAM") as dpool:
            NCHUNK = min(nchunk, dT)
            CM = dT // NCHUNK          # m-tiles per compute chunk
            # G is symmetric: chunk c only stores columns >= kb0(c)*CB.
            # Chunks are PACKED pairwise into NPACK buffers so the AllReduce
            # is 2 calls (collective per-call floor dominates here); packed
            # cols keep their natural kb position minus the pack's base, so
            # pack p covers cols >= pb0(p)*CB and chunk c's unstored
            # (below-base) columns are junk that no reader touches.
            kb0 = [next(kb for kb in range(KB)
                        if (kb + 1) * CB > c * CM * P)
                   for c in range(NCHUNK)]
            assert NCHUNK % npacks == 0
            _pc = NCHUNK // npacks
            PACKS = tuple(tuple(range(p * _pc, (p + 1) * _pc))
                          for p in range(npacks))
            pack_of = {c: p for p, cs in enumerate(PACKS) for c in cs}
            pos_in = {c: i for cs in PACKS for i, c in enumerate(cs)}
            pb0 = [kb0[cs[0]] for cs in PACKS]
            _ashr = "Local" if mock_coll else "Shared"
            g_in = [dpool.tile([len(cs) * CM * P, (KB - pb0[p]) * CB], FP32,
                               name=f"g_in{p}") for p, cs in enumerate(PACKS)]
            g_out = [dpool.tile([len(cs) * CM * P, (KB - pb0[p]) * CB], FP32,
                                name=f"g_out{p}",
                                addr_space=_ashr)
                     for p, cs in enumerate(PACKS)]
            pt_in = [dpool.tile([D // 2, JS], FP16, name=f"pt_in{h}")
                     for h in range(2)]
            pt_out = [dpool.tile([NC, D // 2, JS], FP16, name=f"pt_out{h}",
                                 addr_space=_ashr) for h in range(2)]
            v_park = dpool.tile([D, NL], FP16, name="v_park")

            with tc.tile_pool(name="idt", bufs=1) as idt_pool, \
                 tc.tile_pool(name="stats", bufs=1) as stats_pool, \
                 tc.tile_pool(name="rt", bufs=1) as rt_pool:

                identT = idt_pool.tile([P, P], FP16, name="identT")
                make_identity(nc, identT)
                ident32f = idt_pool.tile([P, P], FP32, name="ident32f")
                make_identity(nc, ident32f)
                nident = idt_pool.tile([P, P], FP32, name="nident")
                make_identity(nc, nident)
                nc.scalar.mul(nident, nident, float(NL))
                ident32 = idt_pool.tile([P, P], FP32R, name="ident32")
                nc.scalar.copy(ident32, ident32f)
                mx = stats_pool.tile([P, IT], FP32, name="mx")
                negm = stats_pool.tile([P, IT], FP32, name="negm")
                ssum = stats_pool.tile([P, IT], FP32, name="ssum")
                recip = stats_pool.tile([P, IT], FP32, name="recip")

                # ---------------- Phase 1: G = X X^T (fp32r), then V -------
                with tc.tile_pool(name="xt", bufs=1) as xt_pool, \
                     tc.tile_pool(name="xn", bufs=1) as xn_pool, \
                     tc.tile_pool(name="vsb", bufs=1) as v_pool:
                    v_sb = [v_pool.tile([P, NL], FP16, name=f"v{iv}")
                            for iv in range(dT)]

                    xts = []
                    for n in range(nT):
                        t = xt_pool.tile([P, D], FP32R, name=f"xt{n}")
                        nc.sync.dma_start(out=t,
                                          in_=xt[n * P:(n + 1) * P, :])
                        xts.append(t)

                    xn_sb = []
                    for k in range(dT):
                        t = xn_pool.tile([P, NL], FP16, name=f"xn{k}")
                        nc.sync.dma_start(out=t, in_=xn_hi[k * P:(k + 1) * P, :])
                        xn_sb.append(t)

                    # G matmuls: G[m,k] = sum_n XT[n,m] * XT[n,k]
                    with tc.tile_pool(name="gstg", bufs=8) as gstg_pool, \
                         tc.tile_pool(name="wk", bufs=4) as wk_pool:
                      wk_pre = []
                      if stop_after not in ("g", "ar"):
                          for k in range(2):
                              wt = wk_pool.tile([P, 4 * P], FP16,
                                                name="wk_t", tag="wk_t")
                              nc.scalar.dma_start(out=wt, in_=wkt_hi[0, k])
                              wk_pre.append(wt)
                      with tc.tile_pool(name="gps", bufs=8,
                                        space="PSUM") as gps_pool:
                        for m in range(dT):
                            ms = slice(m * P, (m + 1) * P)
                            c = m // CM
                            pk = pack_of[c]
                            prow0 = (pos_in[c] * CM + m % CM) * P
                            msl = slice(prow0, prow0 + P)
                            for kb in range(kb0[c], KB):
                                ks = slice(kb * CB, (kb + 1) * CB)
                                ps = gps_pool.tile([P, CB], FP32, name="g_ps",
                                                   tag="g_ps")
                                for n in range(nT):
                                    nc.tensor.matmul(ps, xts[n][:, ms],
                                                     xts[n][:, ks],
                                                     start=(n == 0),
                                                     stop=(n == nT - 1))
                                stg = gstg_pool.tile([P, CB], FP32,
                                                     name="g_stg", tag="g_stg")
                                nc.scalar.copy(stg, ps)
                                if kb == m // 4:
                                    do = (m % 4) * P
                                    nc.vector.tensor_sub(
                                        stg[:, do:do + P], ps[:, do:do + P],
                                        nident)
                                kpk = slice((kb - pb0[pk]) * CB,
                                            (kb - pb0[pk] + 1) * CB)
                                nc.sync.dma_start(out=g_in[pk][msl, kpk],
                                                  in_=stg)
                            # AllReduce a pack as soon as its rows are done
                            if (m % CM == CM - 1 and c == PACKS[pk][-1]
                                    and stop_after not in ("g",)):
                                if mock_coll:
                                    nc.sync.dma_start(out=g_out[pk][:, :],
                                                      in_=g_in[pk][:, :])
                                else:
                                    nc.gpsimd.collective_compute(
                                        "AllReduce", mybir.AluOpType.add,
                                        replica_groups=[list(range(NC))],
                                        ins=[g_in[pk].opt()],
                                        outs=[g_out[pk].opt()])

                      # V = W_k @ X (single-pass fp16), k-outer streamed WkT
                      with tc.tile_pool(name="vps", bufs=4 * NB,
                                        space="PSUM") as vps_pool:
                          for ivg in range(0, dT if stop_after not in
                                           ("g", "ar") else 0, 4):
                            pss = {}
                            for j in range(4):
                                for nb in range(NB):
                                    pss[(j, nb)] = vps_pool.tile(
                                        [P, NBS], FP32, name="v_ps",
                                        tag="v_ps")
                            for k in range(dT):
                                if ivg == 0 and k < len(wk_pre):
                                    wt = wk_pre[k]
                                else:
                                    wt = wk_pool.tile([P, 4 * P], FP16,
                                                      name="wk_t", tag="wk_t")
                                    nc.scalar.dma_start(
                                        out=wt, in_=wkt_hi[ivg // 4, k])
                                for j in range(4):
                                    for nb in range(NB):
                                        ns = slice(nb * NBS, (nb + 1) * NBS)
                                        nc.tensor.matmul(
                                            pss[(j, nb)],
                                            wt[:, j * P:(j + 1) * P],
                                            xn_sb[k][:, ns],
                                            start=(k == 0),
                                            stop=(k == dT - 1))
                            for j in range(4):
                                iv = ivg + j
                                for nb in range(NB):
                                    ns = slice(nb * NBS, (nb + 1) * NBS)
                                    nc.vector.tensor_copy(
                                        out=v_sb[iv][:, ns],
                                        in_=pss[(j, nb)])
                    for iv in range(dT if stop_after not in ("g", "ar")
                                    else 0):
                        nc.sync.dma_start(
                            out=v_park[iv * P:(iv + 1) * P, :],
                            in_=v_sb[iv])

                if stop_after in ("g", "ar", "v"):
                    # still need an output write so a_out is defined
                    dummy = rt_pool.tile([P, NL], FP32, name="dummy_out")
                    nc.vector.memset(dummy, 0.0)
                    for i in range(dT):
                        nc.sync.dma_start(out=a_out[i * P:(i + 1) * P, :],
                                          in_=dummy)
                else:
                    # -------- Phase 2: Rt = G @ (W_q^T/sqrt(d))[:, i_c] ----
                    rts = []
                    with tc.tile_pool(name="gsb", bufs=1) as g_pool, \
                         tc.tile_pool(name="wq", bufs=1) as wq_pool:

                        wqt = []
                        for k in range(dT):
                            h = wq_pool.tile([P, JS], FP32R, name=f"wq{k}")
                            nc.scalar.dma_start(
                                out=h, in_=wqts[k * P:(k + 1) * P, :])
                            wqt.append(h)

                        g_sb = []
                        with tc.tile_pool(name="mirps", bufs=4,
                                          space="PSUM") as mir_pool:
                            for k in range(dT):
                                ck = k // CM
                                pk = pack_of[ck]
                                nsto = D - kb0[ck] * CB
                                pcol = (kb0[ck] - pb0[pk]) * CB
                                prow = (pos_in[ck] * CM + k % CM) * P
                                g = g_pool.tile([P, D], FP32R, name=f"g{k}")
                                nc.scalar.dma_start(
                                    out=g[:, D - nsto:],
                                    in_=g_out[pk][prow:prow + P,
                                                  pcol:pcol + nsto]
                                    .bitcast(FP32R))
                                # mirror below-diagonal: G[k,q] = G[q,k]^T
                                kcs = slice(k * P, (k + 1) * P)
                                for q in range(kb0[ck] * CB // P):
                                    qcs = slice(q * P, (q + 1) * P)
                                    mp = mir_pool.tile([P, P], FP32R,
                                                       name="mir_ps",
                                                       tag="mir_ps")
                                    nc.tensor.transpose(mp[:, :],
                                                        g_sb[q][:, kcs],
                                                        ident32[:, :])
                                    nc.vector.tensor_copy(out=g[:, qcs],
                                                          in_=mp)
                                g_sb.append(g)

                        # Rt[m, i] = sum_k G[k, m] * wqts[k, i]
                        MG = min(8, dT)
                        with tc.tile_pool(name="tps", bufs=MG,
                                          space="PSUM") as tps_pool:
                            for mg in range(0, dT, MG):
                                pss = []
                                for m in range(mg, mg + MG):
                                    pss.append(tps_pool.tile(
                                        [P, JS], FP32, name="t_ps",
                                        tag="t_ps"))
                                for k in range(dT):
                                    for j, m in enumerate(
                                            range(mg, mg + MG)):
                                        ms = slice(m * P, (m + 1) * P)
                                        nc.tensor.matmul(
                                            pss[j], g_sb[k][:, ms],
                                            wqt[k][:, :],
                                            start=(k == 0),
                                            stop=(k == dT - 1))
                                for j, m in enumerate(range(mg, mg + MG)):
                                    h = rt_pool.tile([P, JS], FP32R,
                                                     name=f"rt{m}")
                                    nc.scalar.copy(h, pss[j])
                                    rts.append(h)

                    # -------- Phase 3: S[i_c, :] = Rt^T @ W_v^T ------------
                    with tc.tile_pool(name="s32", bufs=1) as s32_pool, \
                         tc.tile_pool(name="psb", bufs=1) as p_pool, \
                         tc.tile_pool(name="wv", bufs=2) as wv_pool, \
                         tc.tile_pool(name="v2", bufs=1) as v2_pool, \
                         tc.tile_pool(name="sps", bufs=4,
                                      space="PSUM") as sps_pool:
                        s_sb = [s32_pool.tile([P, D], FP32, name=f"s{it}")
                                for it in range(IT)]
                        for it in range(IT):
                            nc.scalar.dma_start(
                                out=s_sb[it],
                                in_=t1s[it * P:(it + 1) * P, :])
                        for jb in range(KB if stop_after != "rt" else 0):
                            wv_t = wv_pool.tile([P, dT * CB], FP32R,
                                                name="wv_t", tag="wv_t")
                            nc.scalar.dma_start(out=wv_t, in_=wvs[jb])
                            for it in range(IT):
                                isl = slice(it * P, (it + 1) * P)
                                ps = sps_pool.tile([P, CB], FP32, name="s_ps",
                                                   tag="s_ps")
                                for m in range(dT):
                                    msl = slice(m * CB, (m + 1) * CB)
                                    nc.tensor.matmul(
                                        ps, rts[m][:, isl],
                                        wv_t[:, msl],
                                        start=(m == 0),
                                        stop=(m == dT - 1))
                                jsl = slice(jb * CB, (jb + 1) * CB)
                                nc.vector.tensor_add(
                                    s_sb[it][:, jsl], ps, s_sb[it][:, jsl])

                        v_sb2 = []
                        if stop_after not in ("rt", "s", "ag"):
                            for iv in range(dT):
                                t = v2_pool.tile([P, NL], FP16,
                                                 name=f"v2_{iv}")
                                nc.scalar.dma_start(
                                    out=t,
                                    in_=v_park[iv * P:(iv + 1) * P, :])
                                v_sb2.append(t)

                        # -------- Phase 4: softmax rows + P^T + AllGather --
                        if stop_after not in ("rt", "s"):
                            with tc.tile_pool(name="ptl", bufs=4) as ptl_pool, \
                                 tc.tile_pool(name="ptps", bufs=4,
                                              space="PSUM") as ptps_pool:
                                pn = []
                                for it in range(IT):
                                    itc = slice(it, it + 1)
                                    nc.vector.reduce_max(
                                        mx[:, itc], s_sb[it],
                                        axis=mybir.AxisListType.X)
                                    nc.scalar.mul(negm[:, itc], mx[:, itc],
                                                  -1.0)
                                    pt = p_pool.tile([P, D], FP16,
                                                     name=f"p{it}")
                                    nc.scalar.activation(
                                        pt, s_sb[it], AF.Exp,
                                        bias=negm[:, itc], scale=1.0,
                                        accum_out=ssum[:, itc])
                                    nc.vector.reciprocal(recip[:, itc],
                                                         ssum[:, itc])
                                    pnt = p_pool.tile([P, D], FP16,
                                                      name=f"pn{it}")
                                    nc.vector.tensor_scalar_mul(
                                        pnt, pt, recip[:, itc])
                                    pn.append(pnt)
                                for half in range(2):
                                    for jt in range(half * dT // 2,
                                                    (half + 1) * dT // 2):
                                        jcs = slice(jt * P, (jt + 1) * P)
                                        jloc = jt - half * dT // 2
                                        ptl = ptl_pool.tile([P, JS], FP16,
                                                            name="ptl",
                                                            tag="ptl")
                                        for it in range(IT):
                                            mp = ptps_pool.tile([P, P], FP16,
                                                                name="pt_ps",
                                                                tag="pt_ps")
                                            nc.tensor.transpose(
                                                mp, pn[it][:, jcs], identT)
                                            nc.vector.tensor_copy(
                                                out=ptl[:,
                                                        it * P:(it + 1) * P],
                                                in_=mp)
                                        nc.sync.dma_start(
                                            out=pt_in[half][
                                                jloc * P:(jloc + 1) * P, :],
                                            in_=ptl)
                                    if mock_coll:
                                        for rr in range(NC):
                                            nc.sync.dma_start(
                                                out=pt_out[half][rr, :, :],
                                                in_=pt_in[half][:, :])
                                    else:
                                        nc.gpsimd.collective_compute(
                                            "AllGather",
                                            mybir.AluOpType.bypass,
                                            replica_groups=[list(range(NC))],
                                            ins=[pt_in[half].opt()],
                                            outs=[pt_out[half].opt()])

                    # -------- Phase 5: A = P @ V (transpose-free) ----------
                    if stop_after not in ("rt", "s", "ag"):
                        with tc.tile_pool(name="ptb", bufs=2) as ptb_pool, \
                             tc.tile_pool(name="asb", bufs=2) as a_pool, \
                             tc.tile_pool(name="aps", bufs=3,
                                          space="PSUM") as aps_pool:
                            v_sb = v_sb2
                            for rr in range(NC):
                                ptb = []
                                for jt in range(dT):
                                    half = jt // (dT // 2)
                                    jloc = jt - half * dT // 2
                                    t = ptb_pool.tile([P, JS], FP16,
                                                      name="ptb",
                                                      tag=f"ptb{jt}")
                                    nc.scalar.dma_start(
                                        out=t,
                                        in_=pt_out[half][
                                            rr, jloc * P:(jloc + 1) * P, :])
                                    ptb.append(t)
                                for isub in range(IT):
                                    i = rr * IT + isub
                                    isl = slice(isub * P, (isub + 1) * P)
                                    aps = aps_pool.tile([P, NL], FP32,
                                                        name="a_ps",
                                                        tag="a_ps")
                                    for jt in range(dT):
                                        for nb in range(NB):
                                            ns = slice(nb * NBS,
                                                       (nb + 1) * NBS)
                                            nc.tensor.matmul(
                                                aps[:, ns], ptb[jt][:, isl],
                                                v_sb[jt][:, ns],
                                                start=(jt == 0),
                                                stop=(jt == dT - 1))
                                    asb = a_pool.tile([P, NL], FP32,
                                                      name="a_sb", tag="a_sb")
                                    nc.vector.tensor_copy(out=asb, in_=aps)
                                    nc.sync.dma_start(
                                        out=a_out[i * P:(i + 1) * P, :],
                                        in_=asb)

    nc.compile()
    return nc


def prepare_inputs(X_t, W_q, W_k, W_v, NC=NCORES):
    """Host-side sharding + layout packing.  Returns in_maps for SPMD."""
    D, N = X_t.shape
    NL = N // NC
    JS = D // NC
    sc = np.float32(1.0) / np.sqrt(np.float32(D))
    dT = D // 128
    P_ = 128
    CB = 512
    KB = D // CB

    wkt_hi = np.ascontiguousarray(W_k.T.astype(np.float16))
    wkt_hi = np.ascontiguousarray(
        wkt_hi.reshape(dT, P_, dT // 4, 4 * P_).transpose(2, 0, 1, 3))

    wqts_full = np.ascontiguousarray(W_q.T.astype(np.float32) * sc)
    wvt = W_v.T.astype(np.float32)            # [D, D] = Wv^T
    # wvs[jb][p, m*CB+j] = Wv^T[m*128+p, jb*CB+j]
    wvs = np.ascontiguousarray(
        wvt.reshape(dT, P_, KB, CB).transpose(2, 1, 0, 3)
        .reshape(KB, P_, dT * CB))

    # weight folding: S = Wq G Wv^T sc = Wq (G - N I) Wv^T sc + N sc Wq Wv^T;
    # the data-independent second term is precomputed here (host), the device
    # computes only the (G - N I) chain.
    t1_full = (np.float32(N) * wqts_full.T @ W_v.T.astype(np.float32))
    in_maps = []
    for c in range(NC):
        xc = np.ascontiguousarray(X_t[:, c * NL:(c + 1) * NL]
                                  .astype(np.float32))
        in_maps.append({
            "xt": np.ascontiguousarray(xc.T),
            "xn_hi": np.ascontiguousarray(xc.astype(np.float16)),
            "wkt_hi": wkt_hi,
            "wqts": np.ascontiguousarray(wqts_full[:, c * JS:(c + 1) * JS]),
            "wvs": wvs,
            "t1s": np.ascontiguousarray(t1_full[c * JS:(c + 1) * JS, :]),
        })
    return in_maps


_CACHED_NC = None


def _get_nc():
    global _CACHED_NC
    if _CACHED_NC is None:
        _CACHED_NC = build()
    return _CACHED_NC


def run(X_t, W_q, W_k, W_v, trace=False):
    from concourse.bass_utils import run_bass_kernel_spmd
    nc = _get_nc()
    in_maps = prepare_inputs(X_t, W_q, W_k, W_v)
    res = run_bass_kernel_spmd(nc, in_maps, core_ids=list(range(NCORES)),
                               trace=trace)
    A = np.concatenate([res.results[c]["a_out"] for c in range(NCORES)],
                       axis=1)
    return A, res


def kernel(X_t, W_q, W_k, W_v):
    X_t = np.asarray(X_t)
    W_q = np.asarray(W_q)
    W_k = np.asarray(W_k)
    W_v = np.asarray(W_v)
    A, _ = run(X_t, W_q, W_k, W_v, trace=False)
    return A.astype(np.float32)
